# revision 30
# baseline (speedup 1.0000x reference)
"""Trainium2 Bass kernel for the Involution module (B=8, C=256, H=W=56, K=7).

Strategy (8 NeuronCores, data-parallel over batch):
  Each core processes one batch element.
  - conv1x1+BN+ReLU twice on the PE (bf16, BN folded into weights in numpy).
  - Involution: partitions = (group g:16, kj:7) = 112 lanes.
    x is pre-replicated 7x (kj-shifted copies) host-side -> streamed into
    SBUF in 4 progressive row-chunks so band 0's products start early.
    For each tap row ki: DVE computes products wgt[g,ki*7+kj,hw] * x[(g,i),hw]
    (bf16 tensor_tensor, wgt broadcast over i via stride-0 AP).
    PE reduces over kj with a 0/1 selection matmul, accumulating the 7 ki
    iterations in PSUM. ACT copies PSUM->SBUF (bf16), DMA writes compact
    bf16 output.
"""

import numpy as np
import ml_dtypes

B, C, H, W = 8, 256, 56, 56
K = 7
GC = 16
G = 16
RED = 64
K2 = 49
EPS = 1e-5
HW = H * W            # 3136
PAD = 3
HP = H + 2 * PAD      # 62
WP = W + 2 * PAD      # 62
NB = 14               # bands
BW = HW // NB         # 224 columns per band (4 output rows)
BR = 4                # rows per band
NKJ = 7
NP = G * NKJ          # 112 partitions

# x_rep row-range loads (r0, r1) into one resident SBUF tile laid out
# [GC, rows, W] per partition (i-major keeps the DVE product APs contiguous).
# Band b, tap-row ki reads input rows 4b+ki .. 4b+ki+3; finer granularity up
# front lets band 0 start early, ranges are overlap-free.
CHUNKS = [(0, 6), (6, 10), (10, 14), (14, 22), (22, 30), (30, 46), (46, 62)]

bf16 = ml_dtypes.bfloat16

_CACHE = {}


def _build_nc():
    import concourse.bacc as bacc
    import concourse.tile as tile
    from concourse import mybir

    f32 = mybir.dt.float32
    b16 = mybir.dt.bfloat16

    nc = bacc.Bacc("TRN2", target_bir_lowering=False, debug=False, num_devices=8)

    # per-chunk x_rep tensors, laid out exactly like their SBUF destination so
    # each partition is one contiguous DMA run
    x_rep = [
        nc.dram_tensor(
            f"x_rep{c}", [NP, (r1 - r0) * GC * W], b16, kind="ExternalInput"
        ).ap()
        for c, (r0, r1) in enumerate(CHUNKS)
    ]
    x_conv = nc.dram_tensor("x_conv", [128, 2, HW], b16, kind="ExternalInput").ap()
    w1t = nc.dram_tensor("w1t", [128, 2, RED], b16, kind="ExternalInput").ap()
    w2t = nc.dram_tensor("w2t", [RED, K, NP], b16, kind="ExternalInput").ap()
    sel = nc.dram_tensor("sel", [NP, G], b16, kind="ExternalInput").ap()
    b1 = nc.dram_tensor("b1", [RED, 1], f32, kind="ExternalInput").ap()
    b2 = nc.dram_tensor("b2", [NP, K], f32, kind="ExternalInput").ap()
    # out compact bf16: [j, g, i4, hw]; channel = 16g + 4j + i4
    out = nc.dram_tensor("out", [4, G, 4, HW], b16, kind="ExternalOutput").ap()

    with tile.TileContext(nc) as tc:
        _body(tc, nc, mybir, x_rep, x_conv, w1t, w2t, sel, b1, b2, out)

    nc.compile()
    return nc


def _body(tc, nc, mybir, x_rep, x_conv, w1t, w2t, sel, b1, b2, out):
    f32 = mybir.dt.float32
    b16 = mybir.dt.bfloat16
    Relu = mybir.ActivationFunctionType.Relu
    mult = mybir.AluOpType.mult

    import contextlib
    ctx = contextlib.ExitStack()
    const = ctx.enter_context(tc.tile_pool(name="const", bufs=1))
    xrp = ctx.enter_context(tc.tile_pool(name="xrp", bufs=1))
    h1p = ctx.enter_context(tc.tile_pool(name="h1p", bufs=3))
    wgp = ctx.enter_context(tc.tile_pool(name="wgp", bufs=3))
    prp = ctx.enter_context(tc.tile_pool(name="prp", bufs=2))
    osp = ctx.enter_context(tc.tile_pool(name="osp", bufs=3))
    ph1 = ctx.enter_context(tc.tile_pool(name="ph1", bufs=2, space="PSUM"))
    ph2 = ctx.enter_context(tc.tile_pool(name="ph2", bufs=2, space="PSUM"))
    pho = ctx.enter_context(tc.tile_pool(name="pho", bufs=2, space="PSUM"))

    # ---- resident constants + all conv input, queued first ----
    # everything the band loop needs from HBM besides x_rep chunks is loaded
    # upfront: any DMA enqueued later would sit behind megabytes of chunk
    # traffic in the queues (strict FIFO) and stall its consumer.
    w1s = const.tile([128, 2, RED], b16, tag="w1s")
    nc.sync.dma_start(out=w1s[:], in_=w1t)
    b1s = const.tile([RED, 1], f32, tag="b1s")
    nc.sync.dma_start(out=b1s[:], in_=b1)

    xcs = const.tile([128, 2, HW], b16, tag="xcs")
    nc.sync.dma_start(out=xcs[:, :, 0:2 * BW], in_=x_conv[:, :, 0:2 * BW])

    # one resident x tile, [GC, rows, W] per partition; row-range DMAs fill it
    # progressively and subtile deps gate each band on just its rows
    xfull = xrp.tile([NP, GC, HP, W], b16, tag="xfull")

    def load_chunk(cidx):
        r0, r1 = CHUNKS[cidx]
        nc.sync.dma_start(
            out=xfull[:, :, r0:r1, :],
            in_=x_rep[cidx].rearrange("p (i r n) -> p i r n", i=GC, r=r1 - r0),
        )

    load_chunk(0)

    w2s = const.tile([RED, K, NP], b16, tag="w2s")
    nc.sync.dma_start(out=w2s[:], in_=w2t)
    sels = const.tile([NP, G], b16, tag="sels")
    nc.sync.dma_start(out=sels[:], in_=sel)
    b2s = const.tile([NP, K], f32, tag="b2s")
    nc.sync.dma_start(out=b2s[:], in_=b2)

    load_chunk(1)
    load_chunk(2)
    load_chunk(3)
    nc.sync.dma_start(out=xcs[:, :, 2 * BW:HW], in_=x_conv[:, :, 2 * BW:HW])
    for cidx in range(4, len(CHUNKS)):
        load_chunk(cidx)

    for b in range(NB):

        n0 = b * BW
        # conv1: h1 = relu(W1' @ x + b1')
        p1 = ph1.tile([RED, BW], f32, tag="p1")
        nc.tensor.matmul(p1[:], w1s[:, 0, :], xcs[:, 0, n0:n0 + BW], start=True, stop=False)
        nc.tensor.matmul(p1[:], w1s[:, 1, :], xcs[:, 1, n0:n0 + BW], start=False, stop=True)
        h1b = h1p.tile([RED, BW], b16, tag="h1b")
        nc.scalar.activation(h1b[:], p1[:], Relu, bias=b1s[:], scale=1.0)

        # conv2 per ki: wgt[(g,kj), ki, hw_band] = relu(W2'[ki] @ h1 + b2'[ki])
        wgb = wgp.tile([NP, K, BW], b16, tag="wgb")
        for ki in range(K):
            p2 = ph2.tile([NP, BW], f32, tag="p2")
            nc.tensor.matmul(p2[:], w2s[:, ki, :], h1b[:], start=True, stop=True)
            nc.scalar.activation(
                wgb[:, ki, :], p2[:], Relu, bias=b2s[:, ki:ki + 1], scale=1.0
            )

        # involution: products + kj/ki reduction.
        # Band 0 runs one DVE op per tap row so it can start as soon as the
        # first rows land; later bands merge all 7 tap rows into one DVE op
        # per group-channel half (sliding-window AP), saving per-op overhead.
        po = pho.tile([128, 2, 512], f32, tag="po")  # s-slot padded to one PSUM bank
        if b == 0:
            for ki in range(K):
                r = BR * b + ki
                pr = prp.tile([NP, GC, BW], b16, tag="pr")
                in0 = xfull[:, :, r:r + BR, :].rearrange("p i r n -> p i (r n)")
                in1 = wgb[:, ki, :].unsqueeze(1).broadcast_to([NP, GC, BW])
                nc.vector.tensor_tensor(out=pr[:], in0=in0, in1=in1, op=mult)
                for p4 in range(8):
                    j, s = p4 // 2, p4 % 2
                    nc.tensor.matmul(
                        po[32 * j:32 * j + G, s, 0:2 * BW],
                        sels[:],
                        pr[:, 2 * p4:2 * p4 + 2, :],
                        start=(ki == 0),
                        stop=(ki == K - 1),
                        tile_position=(0, 32 * j),
                    )
        else:
            r0 = BR * b
            base = xfull[:]
            pstride, pnum = base.ap[0]
            for half in range(2):
                ih = 8 * half
                pr = prp.tile([NP, K, 8, BW], b16, tag="prm")
                in0 = type(base)(
                    base.tensor,
                    base.offset + (ih * HP + r0) * W,
                    [[pstride, pnum], [W, K], [HP * W, 8], [1, BW]],
                )
                in1 = wgb[:].unsqueeze(2).broadcast_to([NP, K, 8, BW])
                nc.vector.tensor_tensor(out=pr[:], in0=in0, in1=in1, op=mult)
                for ki in range(K):
                    for q in range(4):
                        p4 = 4 * half + q
                        j, s = p4 // 2, p4 % 2
                        nc.tensor.matmul(
                            po[32 * j:32 * j + G, s, 0:2 * BW],
                            sels[:],
                            pr[:, ki, 2 * q:2 * q + 2, :],
                            start=(ki == 0),
                            stop=(ki == K - 1),
                            tile_position=(0, 32 * j),
                        )

        # PSUM -> SBUF (bf16) -> HBM compact   (ob rows p=32j+g, free (i4=2s+r, hw))
        # last band: split copies across ACT and the now-idle DVE to shrink the tail
        ob = osp.tile([128, 4, BW], b16, tag="ob")
        for j in range(4):
            o_ap = ob[32 * j:32 * j + G, :, :].rearrange("p (s r) n -> p s r n", s=2)
            i_ap = po[32 * j:32 * j + G, :, 0:2 * BW].rearrange("p s (r n) -> p s r n", r=2)
            if b == NB - 1 and j < 2:
                nc.vector.tensor_copy(out=o_ap, in_=i_ap)
            else:
                nc.scalar.copy(out=o_ap, in_=i_ap)
        for j in range(4):
            nc.sync.dma_start(
                out=out[j, :, :, n0:n0 + BW], in_=ob[32 * j:32 * j + G, :, :]
            )

    ctx.close()


def _prep_weights(w1, b1, g1, be1, m1, v1, w2, b2, g2, be2, m2, v2):
    s1 = (g1 / np.sqrt(v1 + EPS)).astype(np.float64)
    W1p = w1.astype(np.float64) * s1[:, None]
    b1p = be1 + (b1 - m1) * (g1 / np.sqrt(v1 + EPS))
    s2 = (g2 / np.sqrt(v2 + EPS)).astype(np.float64)
    W2p = w2.astype(np.float64) * s2[:, None]
    b2p = be2 + (b2 - m2) * (g2 / np.sqrt(v2 + EPS))

    w1t = np.ascontiguousarray(
        W1p.astype(np.float32).T.reshape(2, 128, RED).transpose(1, 0, 2)
    ).astype(bf16)
    # w2t[r, ki, 7g+kj] = W2p[g*49 + ki*7 + kj, r]
    w2t = np.ascontiguousarray(
        W2p.astype(np.float32).reshape(G, K, K, RED).transpose(3, 1, 0, 2).reshape(RED, K, NP)
    ).astype(bf16)
    b2t = np.ascontiguousarray(
        b2p.astype(np.float32).reshape(G, K, K).transpose(0, 2, 1).reshape(NP, K)
    )
    selm = np.repeat(np.eye(G, dtype=np.float32), NKJ, axis=0).astype(bf16)
    return (
        w1t,
        b1p.astype(np.float32).reshape(RED, 1),
        w2t,
        b2t,
        selm,
    )


def _prep_core(xc):
    """xc: [C, H, W] fp32 -> (x_rep chunk list, x_conv bf16 [128,2,HW])

    x_rep layout: [p=(g,kj), row, gc, w] so each row range is contiguous."""
    xpad = np.zeros((C, HP, WP), np.float32)
    xpad[:, PAD:PAD + H, PAD:PAD + W] = xc
    xg = xpad.reshape(G, GC, HP, WP)
    arr = np.empty((G, NKJ, GC, HP, W), np.float32)
    for kj in range(NKJ):
        arr[:, kj] = xg[:, :, :, kj:kj + W]
    x_rep = arr.reshape(NP, GC, HP, W).astype(bf16)
    chunks = [
        np.ascontiguousarray(x_rep[:, :, r0:r1]).reshape(NP, (r1 - r0) * GC * W)
        for (r0, r1) in CHUNKS
    ]
    x_conv = np.ascontiguousarray(
        xc.reshape(2, 128, HW).transpose(1, 0, 2)
    ).astype(bf16)
    return chunks, x_conv


def kernel(x, w1, b1, g1, be1, m1, v1, w2, b2, g2, be2, m2, v2, _profile=False):
    from concourse.bass_utils import run_bass_kernel_spmd

    if "nc" not in _CACHE:
        _CACHE["nc"] = _build_nc()
    nc = _CACHE["nc"]

    x = np.asarray(x, np.float32)
    w1t, b1p, w2t, b2t, selm = _prep_weights(
        np.asarray(w1, np.float32), np.asarray(b1, np.float32),
        np.asarray(g1, np.float32), np.asarray(be1, np.float32),
        np.asarray(m1, np.float32), np.asarray(v1, np.float32),
        np.asarray(w2, np.float32), np.asarray(b2, np.float32),
        np.asarray(g2, np.float32), np.asarray(be2, np.float32),
        np.asarray(m2, np.float32), np.asarray(v2, np.float32),
    )

    in_maps = []
    for c in range(B):
        chunks, x_conv = _prep_core(x[c].reshape(C, H, W))
        im = {
            "x_conv": x_conv,
            "w1t": w1t, "w2t": w2t, "sel": selm, "b1": b1p, "b2": b2t,
        }
        for ci, ch in enumerate(chunks):
            im[f"x_rep{ci}"] = ch
        in_maps.append(im)

    res = run_bass_kernel_spmd(
        nc, in_maps, core_ids=list(range(8)), trace=_profile
    )
    outs = []
    for c in range(B):
        arr = res.results[c]["out"]  # [4, 16, 4, HW] bf16: [j, g, i4, hw]
        arr = arr.astype(np.float32).transpose(1, 0, 2, 3).reshape(C, HW)
        outs.append(arr)
    outp = np.stack(outs, axis=0)
    if _profile:
        _CACHE["last_result"] = res
    return outp.reshape(B, C, H, W).astype(np.float32)
